# revision 1
# baseline (speedup 1.0000x reference)
"""PointPillarScatter on 8 TRN2 cores via PE one-hot matmul.

Scatter -> dense-matmul transform: host packs pillars (sorted by output
column) into 32-slot windows per 128-column tile.  On device, a one-hot
matrix P[slot, col] = (colof[slot] == col) is built with a single Vector
is_equal per 2 tiles (iota constant vs per-slot column offset, empty slots
get -1 so their row is all-zero), then PSUM[col, feat] = P^T @ feats gives
every output element exactly once (fp32 matmul of a 0/1 matrix is exact).

HW constraints found empirically: matmul operands at base partition 64
fault the exec unit (only 0/32 safe), and multiple accumulation groups
per PSUM bank fault.  So tiles rotate over 2 partition blocks {0,32} and
every matmul owns a full PSUM bank (out at bank offset 0).

Sharding: core k owns flat output columns [k*88000, (k+1)*88000) of the
5*140800 (cav, y, x) space; 688 tiles of 128 cols per core.  8 matmuls
(8 banks) per chunk are Act-copied into one SBUF stage tile [128, 512],
then one 256KB DMA out.  Host re-assembles [5, 64, 200, 704].
"""

import numpy as np

import concourse.bass as bass
import concourse.tile as tile
from concourse import mybir
from concourse.bass_utils import run_bass_kernel_spmd

NUM_FEATURES = 64
MAX_CAV = 5
NX, NY = 704, 200
NUM_PIXELS = NY * NX            # 140800
TOTAL = MAX_CAV * NUM_PIXELS    # 704000
N_CORES = 8
CORE_COLS = TOTAL // N_CORES    # 88000 flat columns per core
TILE_COLS = 128
N_TILES = 688                   # 688*128 = 88064 >= 88000
SLOTS = 32                      # max pillars per tile (seed-0 max is 23)
BLKS = N_TILES // 2             # 344: 2 tiles share one is_equal
CHUNKS = N_TILES // 8           # 86: 8 tiles per out-DMA chunk
OUT_W = N_TILES * NUM_FEATURES  # 44032

_PROG = None


def _split_excess_waits(nc, max_waits=1):
    """Walrus enforces tight per-instruction sync-wait encoding limits. Spill
    surplus waits onto single-wait EventSemaphore nops inserted just before
    the offending instruction on the same engine queue (same semantics:
    engine blocks at the nop, then proceeds)."""
    for blk in nc.main_func.blocks:
        i = 0
        while i < len(blk.instructions):
            inst = blk.instructions[i]
            si = inst.sync_info
            if si is None or len(si.on_wait) <= max_waits:
                i += 1
                continue
            waits = list(si.on_wait)
            keep, spill = waits[-max_waits:], waits[:-max_waits]
            for w in spill:
                nop = mybir.InstEventSemaphore(
                    name=f"I-{nc.next_id()}", ins=[], outs=[]
                )
                nop.engine = inst.engine
                nop.sync_info = mybir.SyncInfo(on_wait=[w], on_update=[])
                nc.register_instruction(nop)
                blk.instructions.insert(i, nop)
                i += 1
            si.on_wait = keep
            inst.sync_info = si
            i += 1


def _build_prog():
    f32 = mybir.dt.float32
    nc = bass.Bass()
    # feats: tile t = 2*b+k lives at partitions [32k, 32k+32), free [64b, 64b+64)
    feats = nc.dram_tensor("feats", [64, BLKS * 64], f32, kind="ExternalInput")
    colof = nc.dram_tensor("colof", [64, BLKS], f32, kind="ExternalInput")
    iota = nc.dram_tensor("iota", [64, 128], f32, kind="ExternalInput")
    # out[p, t*64+f] = feature f of tile t's column p
    out = nc.dram_tensor("out", [128, OUT_W], f32, kind="ExternalOutput")

    with tile.TileContext(nc) as tc:
        with (
            tc.tile_pool(name="const", bufs=1) as constp,
            tc.tile_pool(name="pmat", bufs=3) as pmatp,
            tc.tile_pool(name="psum", bufs=8, space="PSUM") as psump,
            tc.tile_pool(name="stage", bufs=3) as stagep,
        ):
            feats_sb = constp.tile([64, BLKS * 64], f32)
            nc.sync.dma_start(feats_sb[:], feats[:])
            colof_sb = constp.tile([64, BLKS], f32)
            nc.sync.dma_start(colof_sb[:], colof[:])
            iota_sb = constp.tile([64, 128], f32)
            nc.sync.dma_start(iota_sb[:], iota[:])

            P = None
            cur_b = -1
            for c in range(CHUNKS):
                st = stagep.tile([128, 512], f32)
                for j in range(8):
                    t = 8 * c + j
                    b, k = t // 2, t % 2
                    if b != cur_b:
                        P = pmatp.tile([64, 128], f32)
                        nc.vector.tensor_tensor(
                            out=P[:],
                            in0=colof_sb[:, b:b + 1].to_broadcast([64, 128]),
                            in1=iota_sb[:],
                            op=mybir.AluOpType.is_equal,
                        )
                        cur_b = b
                    ps = psump.tile([128, 512], f32, space="PSUM")
                    nc.tensor.matmul(
                        out=ps[:, 0:64],
                        lhsT=P[32 * k:32 * (k + 1), :],
                        rhs=feats_sb[32 * k:32 * (k + 1), b * 64:(b + 1) * 64],
                        start=True,
                        stop=True,
                    )
                    nc.scalar.activation(
                        st[:, j * 64:(j + 1) * 64],
                        ps[:, 0:64],
                        mybir.ActivationFunctionType.Copy,
                    )
                nc.sync.dma_start(out[:, c * 512:(c + 1) * 512], st[:])
    _split_excess_waits(nc)
    return nc


def _host_prep(voxel_coords, pillar_features):
    vc = voxel_coords.astype(np.int64)
    flat = vc[:, 0] * NUM_PIXELS + vc[:, 2] * NX + vc[:, 3]
    feats = np.ascontiguousarray(pillar_features, dtype=np.float32)
    core = flat // CORE_COLS
    rem = flat - core * CORE_COLS
    t = rem // TILE_COLS
    cof = rem - t * TILE_COLS
    k = t % 2
    blk = t // 2
    # slot = rank of pillar within its (core, tile) group
    order = np.argsort(flat, kind="stable")
    gid_sorted = (core * N_TILES + t)[order]
    rank_sorted = np.arange(len(flat)) - np.searchsorted(
        gid_sorted, gid_sorted, side="left"
    )
    slot = np.empty(len(flat), np.int64)
    slot[order] = rank_sorted
    assert slot.max() < SLOTS, f"tile overflow: {slot.max() + 1} slots"
    row = k * SLOTS + slot

    iota_arr = np.broadcast_to(
        np.arange(128, dtype=np.float32), (64, 128)
    ).copy()
    in_maps = []
    for cidx in range(N_CORES):
        m = core == cidx
        fa = np.zeros((64, BLKS, 64), np.float32)
        ca = np.full((64, BLKS), -1.0, np.float32)
        ca[row[m], blk[m]] = cof[m]
        fa[row[m], blk[m], :] = feats[m]
        in_maps.append({
            "feats": fa.reshape(64, BLKS * 64),
            "colof": ca,
            "iota": iota_arr,
        })
    return in_maps


def _unshard(core_outs):
    full = np.empty((TOTAL, NUM_FEATURES), np.float32)
    for cidx, o in enumerate(core_outs):       # o: [128, OUT_W]
        r = o.reshape(128, N_TILES, 64).transpose(1, 0, 2)
        r = r.reshape(N_TILES * 128, 64)
        full[cidx * CORE_COLS:(cidx + 1) * CORE_COLS] = r[:CORE_COLS]
    return np.ascontiguousarray(
        full.reshape(MAX_CAV, NUM_PIXELS, NUM_FEATURES)
        .transpose(0, 2, 1)
        .reshape(MAX_CAV, NUM_FEATURES, NY, NX)
    )


def kernel(voxel_coords, pillar_features):
    global _PROG
    if _PROG is None:
        _PROG = _build_prog()
    in_maps = _host_prep(voxel_coords, pillar_features)
    res = run_bass_kernel_spmd(_PROG, in_maps, list(range(N_CORES)))
    return _unshard([r["out"] for r in res.results])

